# revision 26
# baseline (speedup 1.0000x reference)
"""Trainium2 Bass kernel for BayesLinear sampling forward (v2).

Math (per sample b):
    out[b,o] = sum_i (eps_w[b,o,i] * exp(weight_psi)[o,i] + weight_mu[o,i]) * x[b,i]
             + eps_b[b,o] * exp(bias_psi)[o] + bias_mu[o]

Sharding: data-parallel over batch B=1024 across 8 cores (128 samples each);
mu/psi parameters replicated. No collectives needed (forward only).

Per-core plan (memory-bound: the eps_w stream is 128 MB/core, ~382 us at the
358 GB/s per-NC HBM limit — every other engine must stay under that):
  - eps_w[b] DMA'd as [128 part, 4 c, 512 i] with o = 4*P + c (8 KB
    contiguous per partition -> full DMA efficiency), on the sync HWDGE
    ring; all small loads ride the scalar ring so the big stream starts
    immediately.
  - PE builds m[o,i] = exp(psi)[o,i] * x[b,i] via bf16 diag-matmuls
    (1 cyc/row) from contiguous pre-swizzled lhsT tiles.
  - DVE does ONE fused op per sample: a custom mul-cumsum DVE op
    (body = scan(ADD, Src0*Src1)) streaming eps * m over all 2048
    elements/partition.  Its out AP has a stride-0 inner dim, so each
    128-column segment's last write leaves the cumulative sum at
    y_cum[:, b, c] — a segmented reduction in one instruction instead of
    four affine_mul_reduce ops (cuts DVE time/sample from 3.1 us to 2.4 us,
    putting the Vector engine safely under the DMA roofline).
  - Tail: y_eps[c] = y_cum[.,c] - y_cum[.,c-1]; mu-term x @ mu^T + bias
    precomputed during setup; transpose + combine + store.
"""

import sys

sys.path.insert(0, "/opt/trn_rl_repo")

import numpy as np

B, IN, OUT = 1024, 512, 512
NCORES = 8
BL = B // NCORES  # 128 samples per core
NPAIRS = BL // 2

_CACHE = {}

# DveOpSpec.sha of lower(body=scan(ADD, Src0*Src1)) — pinned so compile()
# validates the table bytes we computed at registration time.
_MUL_CUMSUM_SHA = {"v3": "b3fc3e78a862b7eb", "v4": "bc6a002865d48b97"}


def _mul_cumsum_op():
    """Register (once) and return the fused multiply-cumsum DVE op:
    out[p,k] = sum_{k'<=k} in0[p,k']*in1[p,k']."""
    import concourse.dve_ops as D
    from concourse.dve_spec import AluOp, Spec, Src0, Src1, scan

    for op in D.OPS:
        if op.name == "MUL_CUMSUM_ANT":
            return op

    def _ref(in0, in1, s0, s1, imm2):
        p = in0.astype(np.float32) * in1.astype(np.float32)
        flat = p.reshape(p.shape[0], -1)
        return np.cumsum(flat, axis=-1).reshape(in0.shape).astype(np.float32)

    op = D.DveOp(
        "MUL_CUMSUM_ANT",
        Spec(body=scan(AluOp.ADD, Src0 * Src1), reference=_ref),
        subdim=False,
        uops_sha=dict(_MUL_CUMSUM_SHA),
    )
    D.OPS.append(op)
    D.CUSTOM_DVE_SPECS[op.name] = op.spec
    D._SUB_OPCODE_FOR_NAME[op.name] = max(D._SUB_OPCODE_FOR_NAME.values()) + 1
    return op


def build(npairs=NPAIRS):
    from contextlib import ExitStack

    import concourse.bacc as bacc
    import concourse.mybir as mybir
    import concourse.tile as tile

    f32 = mybir.dt.float32
    bf16 = mybir.dt.bfloat16
    Act = mybir.ActivationFunctionType

    cum_op = _mul_cumsum_op()

    nc = bacc.Bacc("TRN2", target_bir_lowering=False, debug=False)

    x_d = nc.dram_tensor("x", [BL, IN], f32, kind="ExternalInput").ap()
    epsw_d = nc.dram_tensor("eps_w", [BL, OUT, IN], f32, kind="ExternalInput").ap()
    epsb_d = nc.dram_tensor("eps_b", [BL, OUT], f32, kind="ExternalInput").ap()
    wmu_d = nc.dram_tensor("weight_mu", [OUT, IN], f32, kind="ExternalInput").ap()
    wpsi_d = nc.dram_tensor("weight_psi", [OUT, IN], f32, kind="ExternalInput").ap()
    bmu_d = nc.dram_tensor("bias_mu", [1, OUT], f32, kind="ExternalInput").ap()
    bpsi_d = nc.dram_tensor("bias_psi", [1, OUT], f32, kind="ExternalInput").ap()
    id_d = nc.dram_tensor("ident", [128, 128], f32, kind="ExternalInput").ap()
    out_d = nc.dram_tensor("out", [BL, OUT], f32, kind="ExternalOutput").ap()

    with tile.TileContext(nc) as tc, ExitStack() as ctx:
        perm = ctx.enter_context(tc.tile_pool(name="perm", bufs=1))
        strm = ctx.enter_context(tc.tile_pool(name="strm", bufs=4))

        # Small loads on the scalar HWDGE ring — the eps stream (sync ring)
        # starts without queuing behind them.
        # two tiny loads on the sync ring first: warm the DGE/SDMA path so
        # the eps stream hits full rate sooner
        warm = strm.tile([128, 64], f32, tag="warm", bufs=2)
        nc.sync.dma_start(warm[:], x_d[:, 0:64])
        warm2 = strm.tile([128, 64], f32, tag="warm", bufs=2)
        nc.sync.dma_start(warm2[:], x_d[:, 64:128])
        # both rings ramp in parallel: pair 0's eps rides the scalar ring
        # (chunked, interleaved with the setup params) while the sync ring
        # ramps with pair 1 onward
        pre_eps = []
        for s in range(2):
            e = strm.tile([128, 4, IN], f32, tag="eps", bufs=18, name=f"eps_0_{s}")
            pre_eps.append(e)
        pre_src = [epsw_d[s].rearrange("(P c) i -> P c i", c=4) for s in range(2)]
        nc.scalar.dma_start(pre_eps[0][:, 0, :], pre_src[0][:, 0, :])
        nc.scalar.dma_start(pre_eps[0][:, 1, :], pre_src[0][:, 1, :])
        ident = perm.tile([128, 128], f32)
        nc.scalar.dma_start(ident[:], id_d)
        nc.scalar.dma_start(pre_eps[0][:, 2, :], pre_src[0][:, 2, :])
        nc.scalar.dma_start(pre_eps[0][:, 3, :], pre_src[0][:, 3, :])
        x_sb = perm.tile([128, IN], f32)
        nc.scalar.dma_start(x_sb[:], x_d)
        nc.scalar.dma_start(pre_eps[1][:, 0:2, :], pre_src[1][:, 0:2, :])
        nc.scalar.dma_start(pre_eps[1][:, 2:4, :], pre_src[1][:, 2:4, :])
        ones1 = perm.tile([1, 128], f32)
        nc.vector.memset(ones1[:], 1.0)
        ident_b = perm.tile([128, 128], bf16)
        nc.scalar.copy(ident_b[:], ident[:])

        ET = [perm.tile([128, OUT], bf16, tag=f"ET{i}", name=f"ET{i}") for i in range(4)]
        ETt = [
            [
                perm.tile([128, 128], bf16, tag=f"ETt{c}_{ic}", name=f"ETt{c}_{ic}")
                for ic in range(4)
            ]
            for c in range(4)
        ]
        muT = [perm.tile([128, OUT], f32, tag=f"muT{i}", name=f"muT{i}") for i in range(4)]
        xT = [perm.tile([128, 128], f32, tag=f"xT{i}", name=f"xT{i}") for i in range(4)]
        y_cum = perm.tile([128, BL, 4], f32)
        y_eps = [perm.tile([128, BL], f32, tag=f"ye{i}", name=f"ye{i}") for i in range(4)]
        f0 = perm.tile([128, OUT], f32)

        # --- setup: transposes, exp, pre-swizzled lhsT ---
        # Priority order: everything the main-loop pipeline needs (xT, ET,
        # ETt) first; tail-only tensors (eps_b, bias rows, mu) after.
        with tc.tile_pool(name="pss", bufs=4, space="PSUM") as pss:
            for ic in range(4):
                tmp = pss.tile([128, 128], f32, tag="pst")
                nc.tensor.transpose(
                    tmp[:], x_sb[:, ic * 128 : (ic + 1) * 128], ident[:]
                )
                nc.scalar.copy(xT[ic][:], tmp[:])
            for t in range(4):
                psi_sb = strm.tile([128, IN], f32, tag="setup_ld")
                nc.scalar.dma_start(psi_sb[:], wpsi_d[t * 128 : (t + 1) * 128, :])
                for ic in range(4):
                    tmp = pss.tile([128, 128], f32, tag="pst")
                    nc.tensor.transpose(
                        tmp[:], psi_sb[:, ic * 128 : (ic + 1) * 128], ident[:]
                    )
                    nc.scalar.activation(
                        ET[ic][:, t * 128 : (t + 1) * 128], tmp[:], Act.Exp
                    )
            # contiguous lhsT tiles (FWL-eligible) gathered from the o=4P+c
            # interleaved ET columns
            for ic in range(4):
                ETv = ET[ic][:].rearrange("k (P c) -> k P c", c=4)
                for c in range(4):
                    nc.scalar.copy(ETt[c][ic][:], ETv[:, :, c])
            # tail-only loads + transposes (scalar ring idles from here on)
            epsb_sb = perm.tile([128, OUT], f32)
            nc.scalar.dma_start(epsb_sb[:], epsb_d)
            brow = perm.tile([1, OUT], f32)
            nc.scalar.dma_start(brow[:], bmu_d)
            prow = perm.tile([1, OUT], f32)
            nc.scalar.dma_start(prow[:], bpsi_d)
            erow = perm.tile([1, OUT], f32)
            nc.scalar.activation(erow[:], prow[:], Act.Exp)
            for t in range(4):
                mu_sb = strm.tile([128, IN], f32, tag="setup_ld")
                nc.scalar.dma_start(mu_sb[:], wmu_d[t * 128 : (t + 1) * 128, :])
                for ic in range(4):
                    tmp = pss.tile([128, 128], f32, tag="pst")
                    nc.tensor.transpose(
                        tmp[:], mu_sb[:, ic * 128 : (ic + 1) * 128], ident[:]
                    )
                    nc.scalar.copy(muT[ic][:, t * 128 : (t + 1) * 128], tmp[:])
            # static tail terms — run during the stream-start window while
            # the eps pipeline is still filling
            with tc.tile_pool(name="stat", bufs=1, space="PSUM") as stat:
                ebias = stat.tile([128, OUT], f32, tag="ebias")
                nc.tensor.matmul(ebias[:], ones1[:], erow[:], start=True, stop=True)
                mub = stat.tile([128, OUT], f32, tag="mub")
                for ic in range(4):
                    nc.tensor.matmul(
                        mub[:], xT[ic][:], muT[ic][:], start=(ic == 0), stop=False
                    )
                nc.tensor.matmul(mub[:], ones1[:], brow[:], start=False, stop=True)
                nc.vector.tensor_mul(f0[:], epsb_sb[:], ebias[:])
                nc.vector.tensor_add(f0[:], f0[:], mub[:])

        # --- main loop over sample pairs ---
        H = BL // 2  # first-half sample count for the pipelined tail

        def _half0_tail(psm):
            """Diffs + transpose + combine + store for samples 0..H-1, run
            mid-loop once their y_cum columns are final.  Borrows one m-ring
            slot for the transposes (PSUM is otherwise full); the one-sample
            pipeline hiccup is absorbed by the eps prefetch cushion."""
            nc.scalar.copy(y_eps[0][:, 0:H], y_cum[:, 0:H, 0])
            for c in range(1, 4):
                nc.vector.tensor_sub(
                    y_eps[c][:, 0:H], y_cum[:, 0:H, c], y_cum[:, 0:H, c - 1]
                )
            mt = psm.tile([128, 4, IN], f32, tag="m", name="tail_h0")
            for c in range(4):
                nc.tensor.transpose(mt[0:H, c, 0:128], y_eps[c][:, 0:H], ident[:])
            fv0 = f0[0:H, :].rearrange("b (P c) -> b P c", c=4)
            for c in range(4):
                nc.vector.tensor_add(fv0[:, :, c], fv0[:, :, c], mt[0:H, c, 0:128])
            # scalar ring: the sync ring is FIFO and owns the eps stream —
            # a store there would head-of-line block all later eps DMAs
            nc.scalar.dma_start(out_d[0:H, :], f0[0:H, :])

        with tc.tile_pool(name="psm", bufs=2, space="PSUM") as psm:
            for p in range(npairs):
                if p == 0:
                    eps_sb = pre_eps  # loaded on the scalar ring during setup
                else:
                    eps_sb = []
                    for s in range(2):
                        e = strm.tile(
                            [128, 4, IN], f32, tag="eps", bufs=18, name=f"eps_{p}_{s}"
                        )
                        src = epsw_d[2 * p + s].rearrange("(P c) i -> P c i", c=4)
                        b = 2 * p + s
                        if b >= BL - 2:
                            # chunked so the final reduces chase the transfers
                            for c in range(4):
                                nc.sync.dma_start(e[:, c, :], src[:, c, :])
                        elif b < 6:
                            # SWDGE path: an independent descriptor generator,
                            # so these overlap the HWDGE rings' startup ramp
                            nc.gpsimd.dma_start(e[:], src)
                        else:
                            nc.sync.dma_start(e[:], src)
                        eps_sb.append(e)
                dp = strm.tile([128, 4, 256], bf16, tag="dp", bufs=2, name=f"dp_{p}")
                for ic in range(4):
                    for s in range(2):
                        b = 2 * p + s
                        nc.scalar.mul(
                            dp[:, ic, s * 128 : (s + 1) * 128],
                            ident_b[:],
                            xT[ic][:, b : b + 1],
                        )
                for s in range(2):
                    b = 2 * p + s
                    m = psm.tile([128, 4, IN], f32, tag="m", name=f"m_{p}_{s}")
                    for c in range(4):
                        for ic in range(4):
                            nc.tensor.matmul(
                                m[:, c, ic * 128 : (ic + 1) * 128],
                                ETt[c][ic][:],
                                dp[:, ic, s * 128 : (s + 1) * 128],
                                start=True,
                                stop=True,
                            )
                    if b >= BL - 2:
                        # two half-reduces chasing the chunked DMA; the second
                        # half's cumsum restarts from zero, so add the first
                        # half's total back to keep the global-cumsum contract
                        for h in range(2):
                            out_ap = (
                                y_cum[:, b, 2 * h : 2 * h + 2]
                                .unsqueeze(2)
                                .broadcast_to([128, 2, IN])
                            )
                            nc.vector._custom_dve(
                                cum_op,
                                out=out_ap,
                                in0=eps_sb[s][:, 2 * h : 2 * h + 2, :].rearrange(
                                    "P c i -> P (c i)"
                                ),
                                in1=m[:, 2 * h : 2 * h + 2, :].rearrange(
                                    "P c i -> P (c i)"
                                ),
                            )
                        nc.vector.tensor_add(
                            y_cum[:, b, 2:4],
                            y_cum[:, b, 2:4],
                            y_cum[:, b, 1:2].broadcast_to([128, 2]),
                        )
                    else:
                        out_ap = y_cum[:, b, :].unsqueeze(2).broadcast_to([128, 4, IN])
                        nc.vector._custom_dve(
                            cum_op,
                            out=out_ap,
                            in0=eps_sb[s][:].rearrange("P c i -> P (c i)"),
                            in1=m[:].rearrange("P c i -> P (c i)"),
                        )
                if 2 * (p + 1) == H:
                    _half0_tail(psm)

        # --- tail (second half): diffs, transpose, combine, store ---
        nc.scalar.copy(y_eps[0][:, H:BL], y_cum[:, H:BL, 0])
        for c in range(1, 4):
            nc.vector.tensor_sub(
                y_eps[c][:, H:BL], y_cum[:, H:BL, c], y_cum[:, H:BL, c - 1]
            )
        with tc.tile_pool(name="psf", bufs=1, space="PSUM") as psf:
            tT = [psf.tile([128, BL], f32, tag=f"tT{c}", name=f"tT{c}") for c in range(4)]
            for c in range(4):
                # full-width transpose: columns 0..H-1 are stale but unused
                nc.tensor.transpose(tT[c][:], y_eps[c][:], ident[:])
            fv1 = f0[H:BL, :].rearrange("b (P c) -> b P c", c=4)
            for c in range(4):
                nc.vector.tensor_add(fv1[:, :, c], fv1[:, :, c], tT[c][H:BL, :])
            nc.scalar.dma_start(out_d[H:BL, :], f0[H:BL, :])

    nc.compile()
    return nc


def _in_maps(x, eps_w, eps_b, weight_mu, weight_psi, bias_mu, bias_psi):
    ident = np.eye(128, dtype=np.float32)
    maps = []
    for c in range(NCORES):
        sl = slice(c * BL, (c + 1) * BL)
        maps.append(
            {
                "x": np.ascontiguousarray(x[sl], dtype=np.float32),
                "eps_w": np.ascontiguousarray(eps_w[sl], dtype=np.float32),
                "eps_b": np.ascontiguousarray(eps_b[sl], dtype=np.float32),
                "weight_mu": np.ascontiguousarray(weight_mu, dtype=np.float32),
                "weight_psi": np.ascontiguousarray(weight_psi, dtype=np.float32),
                "bias_mu": np.ascontiguousarray(
                    bias_mu.reshape(1, OUT), dtype=np.float32
                ),
                "bias_psi": np.ascontiguousarray(
                    bias_psi.reshape(1, OUT), dtype=np.float32
                ),
                "ident": ident,
            }
        )
    return maps


def kernel(x, eps_w, eps_b, weight_mu, weight_psi, bias_mu, bias_psi, **run_kwargs):
    from concourse.bass_utils import run_bass_kernel_spmd

    if "nc" not in _CACHE:
        _CACHE["nc"] = build()
    nc = _CACHE["nc"]
    maps = _in_maps(x, eps_w, eps_b, weight_mu, weight_psi, bias_mu, bias_psi)
    res = run_bass_kernel_spmd(nc, maps, list(range(NCORES)), **run_kwargs)
    out = np.concatenate([r["out"] for r in res.results], axis=0)
    _CACHE["last_results"] = res
    return out


# revision 27
# speedup vs baseline: 1.0023x; 1.0023x over previous
"""Trainium2 Bass kernel for BayesLinear sampling forward (v2).

Math (per sample b):
    out[b,o] = sum_i (eps_w[b,o,i] * exp(weight_psi)[o,i] + weight_mu[o,i]) * x[b,i]
             + eps_b[b,o] * exp(bias_psi)[o] + bias_mu[o]

Sharding: data-parallel over batch B=1024 across 8 cores (128 samples each);
mu/psi parameters replicated. No collectives needed (forward only).

Per-core plan (memory-bound: the eps_w stream is 128 MB/core, ~382 us at the
358 GB/s per-NC HBM limit — every other engine must stay under that):
  - eps_w[b] DMA'd as [128 part, 4 c, 512 i] with o = 4*P + c (8 KB
    contiguous per partition -> full DMA efficiency), on the sync HWDGE
    ring; all small loads ride the scalar ring so the big stream starts
    immediately.
  - PE builds m[o,i] = exp(psi)[o,i] * x[b,i] via bf16 diag-matmuls
    (1 cyc/row) from contiguous pre-swizzled lhsT tiles.
  - DVE does ONE fused op per sample: a custom mul-cumsum DVE op
    (body = scan(ADD, Src0*Src1)) streaming eps * m over all 2048
    elements/partition.  Its out AP has a stride-0 inner dim, so each
    128-column segment's last write leaves the cumulative sum at
    y_cum[:, b, c] — a segmented reduction in one instruction instead of
    four affine_mul_reduce ops (cuts DVE time/sample from 3.1 us to 2.4 us,
    putting the Vector engine safely under the DMA roofline).
  - Tail: y_eps[c] = y_cum[.,c] - y_cum[.,c-1]; mu-term x @ mu^T + bias
    precomputed during setup; transpose + combine + store.
"""

import sys

sys.path.insert(0, "/opt/trn_rl_repo")

import numpy as np

B, IN, OUT = 1024, 512, 512
NCORES = 8
BL = B // NCORES  # 128 samples per core
NPAIRS = BL // 2

_CACHE = {}

# DveOpSpec.sha of lower(body=scan(ADD, Src0*Src1)) — pinned so compile()
# validates the table bytes we computed at registration time.
_MUL_CUMSUM_SHA = {"v3": "b3fc3e78a862b7eb", "v4": "bc6a002865d48b97"}


def _mul_cumsum_op():
    """Register (once) and return the fused multiply-cumsum DVE op:
    out[p,k] = sum_{k'<=k} in0[p,k']*in1[p,k']."""
    import concourse.dve_ops as D
    from concourse.dve_spec import AluOp, Spec, Src0, Src1, scan

    for op in D.OPS:
        if op.name == "MUL_CUMSUM_ANT":
            return op

    def _ref(in0, in1, s0, s1, imm2):
        p = in0.astype(np.float32) * in1.astype(np.float32)
        flat = p.reshape(p.shape[0], -1)
        return np.cumsum(flat, axis=-1).reshape(in0.shape).astype(np.float32)

    op = D.DveOp(
        "MUL_CUMSUM_ANT",
        Spec(body=scan(AluOp.ADD, Src0 * Src1), reference=_ref),
        subdim=False,
        uops_sha=dict(_MUL_CUMSUM_SHA),
    )
    D.OPS.append(op)
    D.CUSTOM_DVE_SPECS[op.name] = op.spec
    D._SUB_OPCODE_FOR_NAME[op.name] = max(D._SUB_OPCODE_FOR_NAME.values()) + 1
    return op


def build(npairs=NPAIRS):
    from contextlib import ExitStack

    import concourse.bacc as bacc
    import concourse.mybir as mybir
    import concourse.tile as tile

    f32 = mybir.dt.float32
    bf16 = mybir.dt.bfloat16
    Act = mybir.ActivationFunctionType

    cum_op = _mul_cumsum_op()

    nc = bacc.Bacc("TRN2", target_bir_lowering=False, debug=False)

    x_d = nc.dram_tensor("x", [BL, IN], f32, kind="ExternalInput").ap()
    epsw_d = nc.dram_tensor("eps_w", [BL, OUT, IN], f32, kind="ExternalInput").ap()
    epsb_d = nc.dram_tensor("eps_b", [BL, OUT], f32, kind="ExternalInput").ap()
    wmu_d = nc.dram_tensor("weight_mu", [OUT, IN], f32, kind="ExternalInput").ap()
    wpsi_d = nc.dram_tensor("weight_psi", [OUT, IN], f32, kind="ExternalInput").ap()
    bmu_d = nc.dram_tensor("bias_mu", [1, OUT], f32, kind="ExternalInput").ap()
    bpsi_d = nc.dram_tensor("bias_psi", [1, OUT], f32, kind="ExternalInput").ap()
    id_d = nc.dram_tensor("ident", [128, 128], f32, kind="ExternalInput").ap()
    out_d = nc.dram_tensor("out", [BL, OUT], f32, kind="ExternalOutput").ap()

    with tile.TileContext(nc) as tc, ExitStack() as ctx:
        perm = ctx.enter_context(tc.tile_pool(name="perm", bufs=1))
        strm = ctx.enter_context(tc.tile_pool(name="strm", bufs=4))

        # Small loads on the scalar HWDGE ring — the eps stream (sync ring)
        # starts without queuing behind them.
        # two tiny loads on the sync ring first: warm the DGE/SDMA path so
        # the eps stream hits full rate sooner
        warm = strm.tile([128, 64], f32, tag="warm", bufs=2)
        nc.sync.dma_start(warm[:], x_d[:, 0:64])
        warm2 = strm.tile([128, 64], f32, tag="warm", bufs=2)
        nc.sync.dma_start(warm2[:], x_d[:, 64:128])
        # both rings ramp in parallel: pair 0's eps rides the scalar ring
        # (chunked, interleaved with the setup params) while the sync ring
        # ramps with pair 1 onward
        pre_eps = []
        for s in range(2):
            e = strm.tile([128, 4, IN], f32, tag="eps", bufs=18, name=f"eps_0_{s}")
            pre_eps.append(e)
        pre_src = [epsw_d[s].rearrange("(P c) i -> P c i", c=4) for s in range(2)]
        nc.scalar.dma_start(pre_eps[0][:, 0, :], pre_src[0][:, 0, :])
        nc.scalar.dma_start(pre_eps[0][:, 1, :], pre_src[0][:, 1, :])
        ident = perm.tile([128, 128], f32)
        nc.scalar.dma_start(ident[:], id_d)
        nc.scalar.dma_start(pre_eps[0][:, 2, :], pre_src[0][:, 2, :])
        nc.scalar.dma_start(pre_eps[0][:, 3, :], pre_src[0][:, 3, :])
        x_sb = perm.tile([128, IN], f32)
        nc.scalar.dma_start(x_sb[:], x_d)
        nc.scalar.dma_start(pre_eps[1][:, 0:2, :], pre_src[1][:, 0:2, :])
        nc.scalar.dma_start(pre_eps[1][:, 2:4, :], pre_src[1][:, 2:4, :])
        ones1 = perm.tile([1, 128], f32)
        nc.vector.memset(ones1[:], 1.0)
        ident_b = perm.tile([128, 128], bf16)
        nc.scalar.copy(ident_b[:], ident[:])

        ET = [perm.tile([128, OUT], bf16, tag=f"ET{i}", name=f"ET{i}") for i in range(4)]
        ETt = [
            [
                perm.tile([128, 128], bf16, tag=f"ETt{c}_{ic}", name=f"ETt{c}_{ic}")
                for ic in range(4)
            ]
            for c in range(4)
        ]
        muT = [perm.tile([128, OUT], f32, tag=f"muT{i}", name=f"muT{i}") for i in range(4)]
        xT = [perm.tile([128, 128], f32, tag=f"xT{i}", name=f"xT{i}") for i in range(4)]
        y_cum = perm.tile([128, BL, 4], f32)
        y_eps = [perm.tile([128, BL], f32, tag=f"ye{i}", name=f"ye{i}") for i in range(4)]
        f0 = perm.tile([128, OUT], f32)

        # --- setup: transposes, exp, pre-swizzled lhsT ---
        # Priority order: everything the main-loop pipeline needs (xT, ET,
        # ETt) first; tail-only tensors (eps_b, bias rows, mu) after.
        with tc.tile_pool(name="pss", bufs=4, space="PSUM") as pss:
            for ic in range(4):
                tmp = pss.tile([128, 128], f32, tag="pst")
                nc.tensor.transpose(
                    tmp[:], x_sb[:, ic * 128 : (ic + 1) * 128], ident[:]
                )
                nc.scalar.copy(xT[ic][:], tmp[:])
            for t in range(4):
                psi_sb = strm.tile([128, IN], f32, tag="setup_ld")
                nc.scalar.dma_start(psi_sb[:], wpsi_d[t * 128 : (t + 1) * 128, :])
                for ic in range(4):
                    tmp = pss.tile([128, 128], f32, tag="pst")
                    nc.tensor.transpose(
                        tmp[:], psi_sb[:, ic * 128 : (ic + 1) * 128], ident[:]
                    )
                    nc.scalar.activation(
                        ET[ic][:, t * 128 : (t + 1) * 128], tmp[:], Act.Exp
                    )
            # contiguous lhsT tiles (FWL-eligible) gathered from the o=4P+c
            # interleaved ET columns
            for ic in range(4):
                ETv = ET[ic][:].rearrange("k (P c) -> k P c", c=4)
                for c in range(4):
                    nc.scalar.copy(ETt[c][ic][:], ETv[:, :, c])
            # tail-only loads + transposes (scalar ring idles from here on)
            epsb_sb = perm.tile([128, OUT], f32)
            nc.scalar.dma_start(epsb_sb[:], epsb_d)
            brow = perm.tile([1, OUT], f32)
            nc.scalar.dma_start(brow[:], bmu_d)
            prow = perm.tile([1, OUT], f32)
            nc.scalar.dma_start(prow[:], bpsi_d)
            erow = perm.tile([1, OUT], f32)
            nc.scalar.activation(erow[:], prow[:], Act.Exp)
            for t in range(4):
                mu_sb = strm.tile([128, IN], f32, tag="setup_ld")
                nc.scalar.dma_start(mu_sb[:], wmu_d[t * 128 : (t + 1) * 128, :])
                for ic in range(4):
                    tmp = pss.tile([128, 128], f32, tag="pst")
                    nc.tensor.transpose(
                        tmp[:], mu_sb[:, ic * 128 : (ic + 1) * 128], ident[:]
                    )
                    nc.scalar.copy(muT[ic][:, t * 128 : (t + 1) * 128], tmp[:])
            # static tail terms — run during the stream-start window while
            # the eps pipeline is still filling
            with tc.tile_pool(name="stat", bufs=1, space="PSUM") as stat:
                ebias = stat.tile([128, OUT], f32, tag="ebias")
                nc.tensor.matmul(ebias[:], ones1[:], erow[:], start=True, stop=True)
                mub = stat.tile([128, OUT], f32, tag="mub")
                for ic in range(4):
                    nc.tensor.matmul(
                        mub[:], xT[ic][:], muT[ic][:], start=(ic == 0), stop=False
                    )
                nc.tensor.matmul(mub[:], ones1[:], brow[:], start=False, stop=True)
                nc.vector.tensor_mul(f0[:], epsb_sb[:], ebias[:])
                nc.vector.tensor_add(f0[:], f0[:], mub[:])

        # --- main loop over sample pairs ---
        H = BL // 2  # first-half sample count for the pipelined tail

        def _half0_tail(psm):
            """Diffs + transpose + combine + store for samples 0..H-1, run
            mid-loop once their y_cum columns are final.  Borrows one m-ring
            slot for the transposes (PSUM is otherwise full); the one-sample
            pipeline hiccup is absorbed by the eps prefetch cushion."""
            nc.scalar.copy(y_eps[0][:, 0:H], y_cum[:, 0:H, 0])
            for c in range(1, 4):
                nc.vector.tensor_sub(
                    y_eps[c][:, 0:H], y_cum[:, 0:H, c], y_cum[:, 0:H, c - 1]
                )
            mt = psm.tile([128, 4, IN], f32, tag="m", name="tail_h0")
            for c in range(4):
                nc.tensor.transpose(mt[0:H, c, 0:128], y_eps[c][:, 0:H], ident[:])
            fv0 = f0[0:H, :].rearrange("b (P c) -> b P c", c=4)
            for c in range(4):
                nc.vector.tensor_add(fv0[:, :, c], fv0[:, :, c], mt[0:H, c, 0:128])
            # scalar ring: the sync ring is FIFO and owns the eps stream —
            # a store there would head-of-line block all later eps DMAs
            nc.scalar.dma_start(out_d[0:H, :], f0[0:H, :])

        with tc.tile_pool(name="psm", bufs=2, space="PSUM") as psm:
            for p in range(npairs):
                if p == 0:
                    eps_sb = pre_eps  # loaded on the scalar ring during setup
                else:
                    eps_sb = []
                    for s in range(2):
                        e = strm.tile(
                            [128, 4, IN], f32, tag="eps", bufs=18, name=f"eps_{p}_{s}"
                        )
                        src = epsw_d[2 * p + s].rearrange("(P c) i -> P c i", c=4)
                        b = 2 * p + s
                        if b >= BL - 2:
                            # chunked so the final reduces chase the transfers
                            for c in range(4):
                                nc.sync.dma_start(e[:, c, :], src[:, c, :])
                        else:
                            nc.sync.dma_start(e[:], src)
                        eps_sb.append(e)
                dp = strm.tile([128, 4, 256], bf16, tag="dp", bufs=2, name=f"dp_{p}")
                for ic in range(4):
                    for s in range(2):
                        b = 2 * p + s
                        nc.scalar.mul(
                            dp[:, ic, s * 128 : (s + 1) * 128],
                            ident_b[:],
                            xT[ic][:, b : b + 1],
                        )
                for s in range(2):
                    b = 2 * p + s
                    m = psm.tile([128, 4, IN], f32, tag="m", name=f"m_{p}_{s}")
                    for c in range(4):
                        for ic in range(4):
                            nc.tensor.matmul(
                                m[:, c, ic * 128 : (ic + 1) * 128],
                                ETt[c][ic][:],
                                dp[:, ic, s * 128 : (s + 1) * 128],
                                start=True,
                                stop=True,
                            )
                    if b >= BL - 2:
                        # two half-reduces chasing the chunked DMA; the second
                        # half's cumsum restarts from zero, so add the first
                        # half's total back to keep the global-cumsum contract
                        for h in range(2):
                            out_ap = (
                                y_cum[:, b, 2 * h : 2 * h + 2]
                                .unsqueeze(2)
                                .broadcast_to([128, 2, IN])
                            )
                            nc.vector._custom_dve(
                                cum_op,
                                out=out_ap,
                                in0=eps_sb[s][:, 2 * h : 2 * h + 2, :].rearrange(
                                    "P c i -> P (c i)"
                                ),
                                in1=m[:, 2 * h : 2 * h + 2, :].rearrange(
                                    "P c i -> P (c i)"
                                ),
                            )
                        nc.vector.tensor_add(
                            y_cum[:, b, 2:4],
                            y_cum[:, b, 2:4],
                            y_cum[:, b, 1:2].broadcast_to([128, 2]),
                        )
                    else:
                        out_ap = y_cum[:, b, :].unsqueeze(2).broadcast_to([128, 4, IN])
                        nc.vector._custom_dve(
                            cum_op,
                            out=out_ap,
                            in0=eps_sb[s][:].rearrange("P c i -> P (c i)"),
                            in1=m[:].rearrange("P c i -> P (c i)"),
                        )
                if 2 * (p + 1) == H:
                    _half0_tail(psm)

        # --- tail (second half): diffs, transpose, combine, store ---
        nc.scalar.copy(y_eps[0][:, H:BL], y_cum[:, H:BL, 0])
        for c in range(1, 4):
            nc.vector.tensor_sub(
                y_eps[c][:, H:BL], y_cum[:, H:BL, c], y_cum[:, H:BL, c - 1]
            )
        with tc.tile_pool(name="psf", bufs=1, space="PSUM") as psf:
            tT = [psf.tile([128, BL], f32, tag=f"tT{c}", name=f"tT{c}") for c in range(4)]
            for c in range(4):
                # full-width transpose: columns 0..H-1 are stale but unused
                nc.tensor.transpose(tT[c][:], y_eps[c][:], ident[:])
            fv1 = f0[H:BL, :].rearrange("b (P c) -> b P c", c=4)
            for c in range(4):
                nc.vector.tensor_add(fv1[:, :, c], fv1[:, :, c], tT[c][H:BL, :])
            nc.scalar.dma_start(out_d[H:BL, :], f0[H:BL, :])

    nc.compile()
    return nc


def _in_maps(x, eps_w, eps_b, weight_mu, weight_psi, bias_mu, bias_psi):
    ident = np.eye(128, dtype=np.float32)
    maps = []
    for c in range(NCORES):
        sl = slice(c * BL, (c + 1) * BL)
        maps.append(
            {
                "x": np.ascontiguousarray(x[sl], dtype=np.float32),
                "eps_w": np.ascontiguousarray(eps_w[sl], dtype=np.float32),
                "eps_b": np.ascontiguousarray(eps_b[sl], dtype=np.float32),
                "weight_mu": np.ascontiguousarray(weight_mu, dtype=np.float32),
                "weight_psi": np.ascontiguousarray(weight_psi, dtype=np.float32),
                "bias_mu": np.ascontiguousarray(
                    bias_mu.reshape(1, OUT), dtype=np.float32
                ),
                "bias_psi": np.ascontiguousarray(
                    bias_psi.reshape(1, OUT), dtype=np.float32
                ),
                "ident": ident,
            }
        )
    return maps


def kernel(x, eps_w, eps_b, weight_mu, weight_psi, bias_mu, bias_psi, **run_kwargs):
    from concourse.bass_utils import run_bass_kernel_spmd

    if "nc" not in _CACHE:
        _CACHE["nc"] = build()
    nc = _CACHE["nc"]
    maps = _in_maps(x, eps_w, eps_b, weight_mu, weight_psi, bias_mu, bias_psi)
    res = run_bass_kernel_spmd(nc, maps, list(range(NCORES)), **run_kwargs)
    out = np.concatenate([r["out"] for r in res.results], axis=0)
    _CACHE["last_results"] = res
    return out
